# revision 49
# baseline (speedup 1.0000x reference)
"""Trainium2 Bass kernel for masked candidate-span attention (ragged_sequence).

Math (per char n):
  q = W @ x_n                         [128]
  s_v = pos_embed[v] . q  for all v   [96]   (dense: sT = (pos@W)^T-chunks @ xT)
  masked softmax over 9 candidates collapses to v-space with multiplicities:
  w_v = cnt_v * exp(s_v), Z = sum w, ctx = (w/Z) @ pos_embed,
  cnt_v = #{c : idx_c == v and mask_c}.
  Rows with no masked-in candidate or l >= seq_len output 0.

v3 pipeline (v-major softmax, gpsimd local_scatter histogram):
  - host pre-transposes x to f16 [DI, NLOC]; re-encodes each char's masked
    candidate multiset as (value, multiplicity) runs (order-free lossless
    re-encoding of (cand_idx, cand_mask)): int16 slot (j%2)*96 + v and f16
    multiplicity, padded with slot=-1 (ignored by the scatter). Output f16.
  - scores sT [96,512] via 4 f16 matmuls; ET = exp(sT) bf16 via ACT.
  - cnt [128, 4x96] f16 via 2 local_scatter ops (Pool) - no tree needed
    since run slots are unique per char.
  - cntT via 4 PE transposes; wT = cntT * ET (bf16).
  - ctx: 4 matmuls stat=wT chunk, mov=pos_aug [96,129] (ones col -> Z).
  - normalize by 1/max(Z,eps) * inlen on ACT; out f16.

Sharding: pure data parallel over batch (2 batches per core x 8 cores).
"""
import os
import sys

import numpy as np

sys.path.insert(0, "/opt/trn_rl_repo")
_HERE = os.path.dirname(os.path.abspath(__file__))
sys.path.insert(0, _HERE)

from contextlib import ExitStack

import concourse.bass as bass  # noqa: E402
import concourse.mybir as mybir  # noqa: E402
from concourse.tile import TileContext  # noqa: E402
from concourse import library_config  # noqa: E402
from concourse.library_overlay import lower_extended_insts  # noqa: E402

# --- walrus workaround: cap sync waits per instruction ---------------------
import concourse.tile as _tile_mod  # noqa: E402
import bass_rust as _br  # noqa: E402
from concourse.vector_clock import ScopedClock  # noqa: E402


def _patched_drain_and_barrier(self, tick_clock, wait_clock):
    nc = self.nc
    probe = mybir.InstNoOp(name=nc.get_next_instruction_name(), ins=[], outs=[])
    probe.engine = mybir.EngineType.SP
    wait_clock.add_sem_waits(probe, ScopedClock({None: tick_clock.global_clock}))
    waits = list(probe.sync_info.on_wait)
    assert self.sems is not None
    by_num = {h.num: h for h in self.sems.allocated().values()}
    for w in waits:
        nc.sync.wait_ge(by_num[w.id], w.wait_value)
    nc.sync.drain()
    nc.all_engine_barrier()
    popped = nc._tile_sem_poison_stack.pop()
    assert popped is self._sem_poison
    nc.clear_and_free_semaphores(list(self.sems.allocated().values()))
    nc.all_engine_barrier()


_tile_mod.TileContext._drain_and_barrier = _patched_drain_and_barrier


def split_excess_waits(nc):
    for f in nc.m.functions:
        for bb in f.blocks:
            out = []
            changed = False
            for inst in bb.instructions:
                si = inst.sync_info
                waits = list(si.on_wait) if si is not None else []
                cap = 2 if isinstance(inst, _br.InstEventSemaphore) else 1
                if len(waits) > cap:
                    excess, keep = waits[:-cap], waits[-cap:]
                    for k in range(0, len(excess), 2):
                        ev = _br.InstEventSemaphore(
                            name=f"{inst.name}-wsplit{k}", ins=[], outs=[])
                        ev.engine = inst.engine
                        ev.sync_info = _br.SyncInfo(on_wait=excess[k:k + 2],
                                                    on_update=[])
                        out.append(ev)
                    inst.sync_info = _br.SyncInfo(on_wait=keep,
                                                  on_update=list(si.on_update))
                    changed = True
                out.append(inst)
            if changed:
                bb.instructions = out


# --- problem constants -----------------------------------------------------
B, L, C = 16, 4096, 9
DI, DO, V = 512, 128, 96
NCORES = 8
BLOC = B // NCORES          # batches per core
NLOC = BLOC * L             # chars per core (8192)
NTILE = NLOC // 128         # 64 char-tiles per core
NSUP = NTILE // 4           # 16 super-tiles (512 chars each)
C10 = 10                    # candidate slots padded to even count
SLOT = 9 * V                # 864 one-hot slots per char

f32 = mybir.dt.float32
f16 = mybir.dt.float16
bf16 = mybir.dt.bfloat16
i32 = mybir.dt.int32
i16 = mybir.dt.int16
Alu = mybir.AluOpType
Act = mybir.ActivationFunctionType
Ax = mybir.AxisListType


def _ap0(ap, free_count):
    """Broadcast a [P,1] AP along free dim with stride 0."""
    return bass.AP(ap.tensor, ap.offset, [ap.ap[0], [0, free_count]])


def build_kernel():
    nc = bass.Bass()
    # x host-packed [128, (pair8, k4, n1024)]: per-pair DMA is contiguous
    x_d = nc.declare_dram_parameter("xT", [128, NSUP // 2 * 4 * 1024], f16,
                                    isOutput=False)
    sidx_d = nc.declare_dram_parameter("sidx", [128, NTILE * C10], i16,
                                       isOutput=False)
    sval_d = nc.declare_dram_parameter("sval", [128, NTILE * C10], f16,
                                       isOutput=False)
    mt_d = nc.declare_dram_parameter("mt", [128, 4 * V], f16, isOutput=False)
    paug_d = nc.declare_dram_parameter("pos_aug", [V, DO + 1], bf16,
                                       isOutput=False)
    # out partition-major [128, (tile64, o128)]: host unshuffles
    out_d = nc.declare_dram_parameter("out", [128, NTILE * DO], f16,
                                      isOutput=True)

    with TileContext(nc) as tc, ExitStack() as es:
        cpool = es.enter_context(tc.tile_pool(name="consts", bufs=1))
        # ---- constants ----
        io_r = cpool.tile([128, 128], i32)
        io_c = cpool.tile([128, 1], i32)
        nc.gpsimd.iota(io_r[:], pattern=[[1, 128]], base=0, channel_multiplier=0)
        nc.gpsimd.iota(io_c[:], pattern=[[0, 1]], base=0, channel_multiplier=1)
        io_rf = cpool.tile([128, 128], f32)
        io_cf = cpool.tile([128, 1], f32)
        nc.vector.tensor_copy(io_rf[:], io_r[:])
        nc.vector.tensor_copy(io_cf[:], io_c[:])
        ident128h = cpool.tile([128, 128], f16)
        nc.vector.tensor_scalar(out=ident128h[:], in0=io_rf[:], scalar1=io_cf[:],
                                scalar2=None, op0=Alu.is_equal)

        # ---- weights (host pre-folded): mt = (pos@W)^T f16, pos_aug bf16 ----
        mt_all = cpool.tile([128, 4 * V], f16)
        nc.sync.dma_start(out=mt_all[:], in_=mt_d[:])
        mt_sb = [mt_all[:, k * V:(k + 1) * V] for k in range(4)]
        pos_aug = cpool.tile([96, DO + 1], bf16)
        nc.sync.dma_start(out=pos_aug[:], in_=paug_d[:])

        # scatter indices + multiplicity values, host-packed in device
        # layout [128, NTILE*10] so the DMA is 128 contiguous rows
        sidx_sb = cpool.tile([128, NTILE * C10], i16)
        nc.scalar.dma_start(out=sidx_sb[:], in_=sidx_d[:])
        sval_sb = cpool.tile([128, NTILE * C10], f16)
        nc.scalar.dma_start(out=sval_sb[:], in_=sval_d[:])

        # switch gpsimd ucode library: no Pool iota/tensor ops after this
        nc.gpsimd.load_library(library_config.local_scatter)

        # ---- pools ----
        xpool = es.enter_context(tc.tile_pool(name="x", bufs=3))
        epool = es.enter_context(tc.tile_pool(name="eq", bufs=4))
        spool = es.enter_context(tc.tile_pool(name="soft", bufs=4))
        opool = es.enter_context(tc.tile_pool(name="outp", bufs=3))
        ps_s = es.enter_context(tc.tile_pool(name="ps_s", bufs=2, space="PSUM"))
        ps_ct = es.enter_context(tc.tile_pool(name="ps_ct", bufs=2, space="PSUM"))
        ps_cx = es.enter_context(tc.tile_pool(name="ps_cx", bufs=2, space="PSUM"))

        for it in range(NSUP // 2):
            # x pair-tile [128, 2sub x 4k x 512n] f16; split only the first
            # pair's DMA so the very first matmul starts sooner
            xs = xpool.tile([128, 4 * 1024], f16, tag="xs")
            if it == 0:
                for sub in range(2):
                    nc.sync.dma_start(
                        out=xs[:, sub * 2048:(sub + 1) * 2048],
                        in_=x_d[:, sub * 2048:(sub + 1) * 2048])
            else:
                nc.sync.dma_start(out=xs[:],
                                  in_=x_d[:, it * 4096:(it + 1) * 4096])
            outsb = opool.tile([128, 1024], f16, tag="outsb")

            for sub in range(2):
                st = it * 2 + sub
                # scores sT [96v, 512n] (PSUM, f32)
                pst = ps_s.tile([96, 512], f32, tag="ps")
                for k in range(4):
                    nc.tensor.matmul(pst[:], mt_sb[k],
                                     xs[:, sub * 2048 + k * 512:
                                        sub * 2048 + (k + 1) * 512],
                                     start=(k == 0), stop=(k == 3))
                # ET = exp(sT) bf16
                et = spool.tile([96, 512], bf16, tag="et")
                nc.scalar.activation(out=et[:], in_=pst[:], func=Act.Exp,
                                     bias=0.0, scale=1.0)

                # cnt [128, 4j x 96v] f16 via one local_scatter per supertile
                cnt = epool.tile([128, 4 * V], f16, tag="cnt")
                nc.gpsimd.local_scatter(
                    cnt[:], sval_sb[:, st * 4 * C10:(st + 1) * 4 * C10],
                    sidx_sb[:, st * 4 * C10:(st + 1) * 4 * C10],
                    channels=128, num_elems=4 * V, num_idxs=40)

                # cntT [96v, 512n] via PE transposes
                pct = ps_ct.tile([96, 512], f16, tag="pc")
                for j in range(4):
                    nc.tensor.transpose(pct[:, j * 128:(j + 1) * 128],
                                        cnt[:, j * V:(j + 1) * V],
                                        ident128h[:])
                # wT = cntT * ET (bf16) straight out of PSUM
                wt = spool.tile([96, 512], bf16, tag="wt")
                nc.vector.tensor_tensor(out=wt[:], in0=pct[:], in1=et[:],
                                        op=Alu.mult)

                # ctx: per j, out [128n, 129] = wT_j^T @ pos_aug (col 128 = Z)
                pcx = []
                for half in range(2):
                    pc = ps_cx.tile([128, 258], f32, tag=f"cx{half}")
                    for jj in range(2):
                        j = half * 2 + jj
                        nc.tensor.matmul(pc[:, jj * 129:(jj + 1) * 129],
                                         wt[:, j * 128:(j + 1) * 128],
                                         pos_aug[:], start=True, stop=True)
                    pcx.append(pc)

                # Z prep: zg [128,4] <- max(Z cols, eps); rzf = 1/zg * inlen
                zg = spool.tile([128, 4], f32, tag="zg")
                for half in range(2):
                    src = pcx[half][:]
                    nc.vector.tensor_scalar(
                        out=zg[:, half * 2:(half + 1) * 2],
                        in0=bass.AP(src.tensor, src.offset + 128,
                                    [src.ap[0], [129, 2]]),
                        scalar1=1e-30, scalar2=None, op0=Alu.max)
                rz = spool.tile([128, 4], f32, tag="rz")
                nc.vector.reciprocal(rz[:], zg[:])

                # normalize (ACT/DVE/Pool/DVE), out f16. Padding chars
                # beyond seq_len have all-zero sval -> Z=0 -> ctxU=0 -> 0.
                for j in range(4):
                    half, jj = divmod(j, 2)
                    src = pcx[half][:, jj * 129:jj * 129 + 128]
                    dst = outsb[:, sub * 512 + j * 128:sub * 512 + (j + 1) * 128]
                    if j % 2 == 0:
                        nc.scalar.activation(out=dst, in_=src, func=Act.Copy,
                                             bias=0.0, scale=rz[:, j:j + 1])
                    else:
                        nc.vector.tensor_scalar(out=dst, in0=src,
                                                scalar1=rz[:, j:j + 1],
                                                scalar2=None, op0=Alu.mult)

            nc.sync.dma_start(out=out_d[:, it * 1024:(it + 1) * 1024],
                              in_=outsb[:])

    split_excess_waits(nc)
    lower_extended_insts(nc)
    return nc


_NC_CACHE = None


def _host_prep(inputs):
    import ml_dtypes

    x = np.ascontiguousarray(inputs["input_context"], dtype=np.float32)
    W = np.ascontiguousarray(inputs["W"], dtype=np.float32)
    pos = np.ascontiguousarray(inputs["pos_embed"], dtype=np.float32)
    idx = np.asarray(inputs["cand_idx"]).astype(np.int16)
    msk = np.asarray(inputs["cand_mask"]).astype(bool)
    slen = np.ascontiguousarray(inputs["word_seq_len"], dtype=np.int32)

    # one-time weight folding: mt = (pos_embed @ W)^T, pos_aug = pos | ones
    mt = (pos @ W).T.astype(np.float16)                             # [DI, V]
    mt = np.ascontiguousarray(
        mt.reshape(4, 128, V).transpose(1, 0, 2).reshape(128, 4 * V))
    pos_aug = np.concatenate([pos, np.ones((V, 1), np.float32)],
                             axis=1).astype(ml_dtypes.bfloat16)     # [V, DO+1]

    # re-encode each char's masked candidate multiset as (value, count)
    # runs: sort, mark run starts, count run lengths. Lossless re-encoding
    # of (cand_idx, cand_mask) since softmax is order-invariant.
    a = np.where(msk, idx, np.int16(127))             # [B, L, 9]
    s = np.sort(a, axis=-1)
    f = np.empty(s.shape, bool)
    f[..., 0] = True
    f[..., 1:] = s[..., 1:] != s[..., :-1]
    cpos = np.arange(C, dtype=np.int16)
    nxt = np.empty(s.shape, np.int16)
    last = np.full(s.shape[:-1], C, np.int16)
    for i in range(C - 1, -1, -1):
        nxt[..., i] = last
        last = np.where(f[..., i], np.int16(i), last)
    runlen = (nxt - cpos[None, None, :]).astype(np.float16)
    valid = f & (s != 127)
    jmod = ((np.arange(L) // 128) % 4).astype(np.int16) * V  # [L]
    slot = np.where(valid, s + jmod[None, :, None], np.int16(-1))
    sidx = np.full((B, L, C10), -1, np.int16)
    sidx[:, :, :C] = slot
    # fold the sequence-length padding mask into the multiplicities:
    # chars at l >= seq_len get all-zero counts -> Z=0 -> output 0
    in_len = (np.arange(L)[None, :] < slen.reshape(B, 1))  # [B, L]
    sval = np.zeros((B, L, C10), np.float16)
    sval[:, :, :C] = np.where(valid & in_len[:, :, None], runlen,
                              np.float16(0))

    in_maps = []
    for c in range(NCORES):
        b0 = c * BLOC
        # [NLOC, DI] -> [DI, NLOC] -> [k4, 128, it8, sub2, n512] ->
        # [128, (it, sub, k, n)] so each half-pair DMA is contiguous
        xT = x[b0:b0 + BLOC].reshape(NLOC, DI).T.astype(np.float16)
        xT = np.ascontiguousarray(
            xT.reshape(4, 128, 8, 2, 512).transpose(1, 2, 3, 0, 4)
            .reshape(128, 8 * 4 * 1024))
        # [NLOC, 10] -> [NTILE, 128, 10] -> [128, NTILE*10] device layout
        sidx_c = np.ascontiguousarray(
            sidx[b0:b0 + BLOC].reshape(NTILE, 128, C10)
            .transpose(1, 0, 2).reshape(128, NTILE * C10))
        sval_c = np.ascontiguousarray(
            sval[b0:b0 + BLOC].reshape(NTILE, 128, C10)
            .transpose(1, 0, 2).reshape(128, NTILE * C10))
        in_maps.append({
            "xT": xT,
            "sidx": sidx_c,
            "sval": sval_c,
            "mt": mt,
            "pos_aug": pos_aug,
        })
    return in_maps


def kernel(**inputs):
    global _NC_CACHE
    from concourse.bass_utils import run_bass_kernel_spmd

    if _NC_CACHE is None:
        _NC_CACHE = build_kernel()
    nc = _NC_CACHE

    in_maps = _host_prep(inputs)
    res = run_bass_kernel_spmd(nc, in_maps, core_ids=list(range(NCORES)))
    out = np.empty((B, L, DO), np.float32)
    for c in range(NCORES):
        # [128, NTILE*DO] -> [NTILE, 128, DO] -> [NLOC, DO]
        oc = res.results[c]["out"].reshape(128, NTILE, DO).transpose(1, 0, 2)
        out[c * BLOC:(c + 1) * BLOC] = (
            oc.astype(np.float32).reshape(BLOC, L, DO))
    return out


# revision 52
# speedup vs baseline: 1.1542x; 1.1542x over previous
"""Trainium2 Bass kernel for masked candidate-span attention (ragged_sequence).

Math (per char n):
  q = W @ x_n                         [128]
  s_v = pos_embed[v] . q  for all v   [96]   (dense: sT = (pos@W)^T-chunks @ xT)
  masked softmax over 9 candidates collapses to v-space with multiplicities:
  w_v = cnt_v * exp(s_v), Z = sum w, ctx = (w/Z) @ pos_embed,
  cnt_v = #{c : idx_c == v and mask_c}.
  Rows with no masked-in candidate or l >= seq_len output 0.

Pipeline (v-major softmax, gpsimd local_scatter histogram):
  - host prep (layout/encoding only, plus one-time weight folding):
    * x transposed/packed to f16 [128, (pair, sub, k, 512)] for contiguous
      per-tile DMAs;
    * each char's masked candidate multiset re-encoded as (value,
      multiplicity) runs - an order-free lossless re-encoding of
      (cand_idx, cand_mask), since softmax is order-invariant: int16 slot
      (j%4)*96 + v and f16 multiplicity, padded with slot=-1 (ignored by
      the scatter); seq-len padding mask folded into the multiplicities;
    * mt = (pos_embed @ W)^T f16 and pos_aug = [pos | ones] bf16.
  - scores sT [96,512] via 4 f16 matmuls; ET = exp(sT) bf16 via ACT.
  - cnt [128, 4x96] f16 via 1 local_scatter per supertile (Pool): run
    slots are unique per char, so no compare/tree-sum is needed at all.
  - cntT via 4 PE transposes (f16 PSUM); wT = cntT * ET (bf16, DVE).
  - ctx: 4 matmuls stat=wT chunk, mov=pos_aug [96,129]; the ones column
    delivers Z = sum(w) in char-major layout for free.
  - normalize by 1/max(Z,eps) split across ACT and DVE; out f16,
    partition-major, host unshuffles + casts to f32.

Sharding: pure data parallel over batch (2 batches per core x 8 cores).
"""
import os
import sys

import numpy as np

sys.path.insert(0, "/opt/trn_rl_repo")
_HERE = os.path.dirname(os.path.abspath(__file__))
sys.path.insert(0, _HERE)

from contextlib import ExitStack

import concourse.bass as bass  # noqa: E402
import concourse.mybir as mybir  # noqa: E402
from concourse.tile import TileContext  # noqa: E402
from concourse import library_config  # noqa: E402
from concourse.library_overlay import lower_extended_insts  # noqa: E402

# --- walrus workaround: cap sync waits per instruction ---------------------
import concourse.tile as _tile_mod  # noqa: E402
import bass_rust as _br  # noqa: E402
from concourse.vector_clock import ScopedClock  # noqa: E402


def _patched_drain_and_barrier(self, tick_clock, wait_clock):
    nc = self.nc
    probe = mybir.InstNoOp(name=nc.get_next_instruction_name(), ins=[], outs=[])
    probe.engine = mybir.EngineType.SP
    wait_clock.add_sem_waits(probe, ScopedClock({None: tick_clock.global_clock}))
    waits = list(probe.sync_info.on_wait)
    assert self.sems is not None
    by_num = {h.num: h for h in self.sems.allocated().values()}
    for w in waits:
        nc.sync.wait_ge(by_num[w.id], w.wait_value)
    nc.sync.drain()
    nc.all_engine_barrier()
    popped = nc._tile_sem_poison_stack.pop()
    assert popped is self._sem_poison
    nc.clear_and_free_semaphores(list(self.sems.allocated().values()))
    nc.all_engine_barrier()


_tile_mod.TileContext._drain_and_barrier = _patched_drain_and_barrier


def split_excess_waits(nc):
    for f in nc.m.functions:
        for bb in f.blocks:
            out = []
            changed = False
            for inst in bb.instructions:
                si = inst.sync_info
                waits = list(si.on_wait) if si is not None else []
                cap = 2 if isinstance(inst, _br.InstEventSemaphore) else 1
                if len(waits) > cap:
                    excess, keep = waits[:-cap], waits[-cap:]
                    for k in range(0, len(excess), 2):
                        ev = _br.InstEventSemaphore(
                            name=f"{inst.name}-wsplit{k}", ins=[], outs=[])
                        ev.engine = inst.engine
                        ev.sync_info = _br.SyncInfo(on_wait=excess[k:k + 2],
                                                    on_update=[])
                        out.append(ev)
                    inst.sync_info = _br.SyncInfo(on_wait=keep,
                                                  on_update=list(si.on_update))
                    changed = True
                out.append(inst)
            if changed:
                bb.instructions = out


# --- problem constants -----------------------------------------------------
B, L, C = 16, 4096, 9
DI, DO, V = 512, 128, 96
NCORES = 8
BLOC = B // NCORES          # batches per core
NLOC = BLOC * L             # chars per core (8192)
NTILE = NLOC // 128         # 64 char-tiles per core
NSUP = NTILE // 4           # 16 super-tiles (512 chars each)
C10 = 10                    # candidate slots padded to even count

f32 = mybir.dt.float32
f16 = mybir.dt.float16
bf16 = mybir.dt.bfloat16
i32 = mybir.dt.int32
i16 = mybir.dt.int16
Alu = mybir.AluOpType
Act = mybir.ActivationFunctionType
Ax = mybir.AxisListType


def build_kernel():
    nc = bass.Bass()
    # x host-packed [128, (pair8, k4, n1024)]: per-pair DMA is contiguous
    x_d = nc.declare_dram_parameter("xT", [128, NSUP // 2 * 4 * 1024], f16,
                                    isOutput=False)
    sidx_d = nc.declare_dram_parameter("sidx", [128, NTILE * C10], i16,
                                       isOutput=False)
    sval_d = nc.declare_dram_parameter("sval", [128, NTILE * C10], f16,
                                       isOutput=False)
    mt_d = nc.declare_dram_parameter("mt", [128, 4 * V], f16, isOutput=False)
    paug_d = nc.declare_dram_parameter("pos_aug", [V, DO + 1], bf16,
                                       isOutput=False)
    # out partition-major [128, (tile64, o128)]: host unshuffles
    out_d = nc.declare_dram_parameter("out", [128, NTILE * DO], f16,
                                      isOutput=True)

    with TileContext(nc) as tc, ExitStack() as es:
        cpool = es.enter_context(tc.tile_pool(name="consts", bufs=1))
        # ---- constants ----
        io_r = cpool.tile([128, 128], i32)
        io_c = cpool.tile([128, 1], i32)
        nc.gpsimd.iota(io_r[:], pattern=[[1, 128]], base=0, channel_multiplier=0)
        nc.gpsimd.iota(io_c[:], pattern=[[0, 1]], base=0, channel_multiplier=1)
        io_rf = cpool.tile([128, 128], f32)
        io_cf = cpool.tile([128, 1], f32)
        nc.vector.tensor_copy(io_rf[:], io_r[:])
        nc.vector.tensor_copy(io_cf[:], io_c[:])
        ident128h = cpool.tile([128, 128], f16)
        nc.vector.tensor_scalar(out=ident128h[:], in0=io_rf[:], scalar1=io_cf[:],
                                scalar2=None, op0=Alu.is_equal)

        # ---- weights (host pre-folded): mt = (pos@W)^T f16, pos_aug bf16 ----
        mt_all = cpool.tile([128, 4 * V], f16)
        nc.sync.dma_start(out=mt_all[:], in_=mt_d[:])
        mt_sb = [mt_all[:, k * V:(k + 1) * V] for k in range(4)]
        pos_aug = cpool.tile([96, DO + 1], bf16)
        nc.sync.dma_start(out=pos_aug[:], in_=paug_d[:])

        # scatter indices + multiplicity values, host-packed in device
        # layout [128, NTILE*10] so the DMA is 128 contiguous rows
        sidx_sb = cpool.tile([128, NTILE * C10], i16)
        nc.scalar.dma_start(out=sidx_sb[:], in_=sidx_d[:])
        sval_sb = cpool.tile([128, NTILE * C10], f16)
        nc.scalar.dma_start(out=sval_sb[:], in_=sval_d[:])

        # switch gpsimd ucode library: no Pool iota/tensor ops after this
        nc.gpsimd.load_library(library_config.local_scatter)

        # ---- pools ----
        xpool = es.enter_context(tc.tile_pool(name="x", bufs=3))
        epool = es.enter_context(tc.tile_pool(name="eq", bufs=4))
        spool = es.enter_context(tc.tile_pool(name="soft", bufs=4))
        opool = es.enter_context(tc.tile_pool(name="outp", bufs=3))
        ps_s = es.enter_context(tc.tile_pool(name="ps_s", bufs=2, space="PSUM"))
        ps_ct = es.enter_context(tc.tile_pool(name="ps_ct", bufs=2, space="PSUM"))
        ps_cx = es.enter_context(tc.tile_pool(name="ps_cx", bufs=2, space="PSUM"))

        for it in range(NSUP // 2):
            # x pair-tile [128, 2sub x 4k x 512n] f16; split only the first
            # pair's DMA so the very first matmul starts sooner
            xs = xpool.tile([128, 4 * 1024], f16, tag="xs")
            if it == 0:
                for sub in range(2):
                    nc.sync.dma_start(
                        out=xs[:, sub * 2048:(sub + 1) * 2048],
                        in_=x_d[:, sub * 2048:(sub + 1) * 2048])
            else:
                nc.sync.dma_start(out=xs[:],
                                  in_=x_d[:, it * 4096:(it + 1) * 4096])
            outsb = opool.tile([128, 1024], f16, tag="outsb")

            for sub in range(2):
                st = it * 2 + sub
                # scores sT [96v, 512n] (PSUM, f32)
                pst = ps_s.tile([96, 512], f32, tag="ps")
                for k in range(4):
                    nc.tensor.matmul(pst[:], mt_sb[k],
                                     xs[:, sub * 2048 + k * 512:
                                        sub * 2048 + (k + 1) * 512],
                                     start=(k == 0), stop=(k == 3))
                # ET = exp(sT) bf16
                et = spool.tile([96, 512], bf16, tag="et")
                nc.scalar.activation(out=et[:], in_=pst[:], func=Act.Exp,
                                     bias=0.0, scale=1.0)

                # cnt [128, 4j x 96v] f16 via one local_scatter per supertile
                cnt = epool.tile([128, 4 * V], f16, tag="cnt")
                nc.gpsimd.local_scatter(
                    cnt[:], sval_sb[:, st * 4 * C10:(st + 1) * 4 * C10],
                    sidx_sb[:, st * 4 * C10:(st + 1) * 4 * C10],
                    channels=128, num_elems=4 * V, num_idxs=40)

                # cntT [96v, 512n] via PE transposes
                pct = ps_ct.tile([96, 512], f16, tag="pc")
                for j in range(4):
                    nc.tensor.transpose(pct[:, j * 128:(j + 1) * 128],
                                        cnt[:, j * V:(j + 1) * V],
                                        ident128h[:])
                # wT = cntT * ET (bf16) straight out of PSUM
                wt = spool.tile([96, 512], bf16, tag="wt")
                nc.vector.tensor_tensor(out=wt[:], in0=pct[:], in1=et[:],
                                        op=Alu.mult)

                # ctx: per j, out [128n, 129] = wT_j^T @ pos_aug (col 128 = Z)
                pcx = []
                for half in range(2):
                    pc = ps_cx.tile([128, 258], f32, tag=f"cx{half}")
                    for jj in range(2):
                        j = half * 2 + jj
                        nc.tensor.matmul(pc[:, jj * 129:(jj + 1) * 129],
                                         wt[:, j * 128:(j + 1) * 128],
                                         pos_aug[:], start=True, stop=True)
                    pcx.append(pc)

                # Z prep: zg [128,4] <- max(Z cols, eps); rzf = 1/zg * inlen
                zg = spool.tile([128, 4], f32, tag="zg")
                for half in range(2):
                    src = pcx[half][:]
                    nc.vector.tensor_scalar(
                        out=zg[:, half * 2:(half + 1) * 2],
                        in0=bass.AP(src.tensor, src.offset + 128,
                                    [src.ap[0], [129, 2]]),
                        scalar1=1e-30, scalar2=None, op0=Alu.max)
                rz = spool.tile([128, 4], f32, tag="rz")
                nc.vector.reciprocal(rz[:], zg[:])

                # normalize (ACT/DVE/Pool/DVE), out f16. Padding chars
                # beyond seq_len have all-zero sval -> Z=0 -> ctxU=0 -> 0.
                for j in range(4):
                    half, jj = divmod(j, 2)
                    src = pcx[half][:, jj * 129:jj * 129 + 128]
                    dst = outsb[:, sub * 512 + j * 128:sub * 512 + (j + 1) * 128]
                    if j % 2 == 0:
                        nc.scalar.activation(out=dst, in_=src, func=Act.Copy,
                                             bias=0.0, scale=rz[:, j:j + 1])
                    else:
                        nc.vector.tensor_scalar(out=dst, in0=src,
                                                scalar1=rz[:, j:j + 1],
                                                scalar2=None, op0=Alu.mult)

            nc.sync.dma_start(out=out_d[:, it * 1024:(it + 1) * 1024],
                              in_=outsb[:])

    split_excess_waits(nc)
    lower_extended_insts(nc)
    return nc


_NC_CACHE = None


def _host_prep(inputs):
    import ml_dtypes

    x = np.ascontiguousarray(inputs["input_context"], dtype=np.float32)
    W = np.ascontiguousarray(inputs["W"], dtype=np.float32)
    pos = np.ascontiguousarray(inputs["pos_embed"], dtype=np.float32)
    idx = np.asarray(inputs["cand_idx"]).astype(np.int16)
    msk = np.asarray(inputs["cand_mask"]).astype(bool)
    slen = np.ascontiguousarray(inputs["word_seq_len"], dtype=np.int32)

    # one-time weight folding: mt = (pos_embed @ W)^T, pos_aug = pos | ones
    mt = (pos @ W).T.astype(np.float16)                             # [DI, V]
    mt = np.ascontiguousarray(
        mt.reshape(4, 128, V).transpose(1, 0, 2).reshape(128, 4 * V))
    pos_aug = np.concatenate([pos, np.ones((V, 1), np.float32)],
                             axis=1).astype(ml_dtypes.bfloat16)     # [V, DO+1]

    # re-encode each char's masked candidate multiset as (value, count)
    # runs: sort, mark run starts, count run lengths. Lossless re-encoding
    # of (cand_idx, cand_mask) since softmax is order-invariant.
    a = np.where(msk, idx, np.int16(127))             # [B, L, 9]
    s = np.sort(a, axis=-1)
    f = np.empty(s.shape, bool)
    f[..., 0] = True
    f[..., 1:] = s[..., 1:] != s[..., :-1]
    cpos = np.arange(C, dtype=np.int16)
    nxt = np.empty(s.shape, np.int16)
    last = np.full(s.shape[:-1], C, np.int16)
    for i in range(C - 1, -1, -1):
        nxt[..., i] = last
        last = np.where(f[..., i], np.int16(i), last)
    runlen = (nxt - cpos[None, None, :]).astype(np.float16)
    valid = f & (s != 127)
    jmod = ((np.arange(L) // 128) % 4).astype(np.int16) * V  # [L]
    slot = np.where(valid, s + jmod[None, :, None], np.int16(-1))
    sidx = np.full((B, L, C10), -1, np.int16)
    sidx[:, :, :C] = slot
    # fold the sequence-length padding mask into the multiplicities:
    # chars at l >= seq_len get all-zero counts -> Z=0 -> output 0
    in_len = (np.arange(L)[None, :] < slen.reshape(B, 1))  # [B, L]
    sval = np.zeros((B, L, C10), np.float16)
    sval[:, :, :C] = np.where(valid & in_len[:, :, None], runlen,
                              np.float16(0))

    in_maps = []
    for c in range(NCORES):
        b0 = c * BLOC
        # [NLOC, DI] -> [DI, NLOC] -> [k4, 128, it8, sub2, n512] ->
        # [128, (it, sub, k, n)] so each half-pair DMA is contiguous
        xT = x[b0:b0 + BLOC].reshape(NLOC, DI).T.astype(np.float16)
        xT = np.ascontiguousarray(
            xT.reshape(4, 128, 8, 2, 512).transpose(1, 2, 3, 0, 4)
            .reshape(128, 8 * 4 * 1024))
        # [NLOC, 10] -> [NTILE, 128, 10] -> [128, NTILE*10] device layout
        sidx_c = np.ascontiguousarray(
            sidx[b0:b0 + BLOC].reshape(NTILE, 128, C10)
            .transpose(1, 0, 2).reshape(128, NTILE * C10))
        sval_c = np.ascontiguousarray(
            sval[b0:b0 + BLOC].reshape(NTILE, 128, C10)
            .transpose(1, 0, 2).reshape(128, NTILE * C10))
        in_maps.append({
            "xT": xT,
            "sidx": sidx_c,
            "sval": sval_c,
            "mt": mt,
            "pos_aug": pos_aug,
        })
    return in_maps


def kernel(**inputs):
    global _NC_CACHE
    from concourse.bass_utils import run_bass_kernel_spmd

    if _NC_CACHE is None:
        _NC_CACHE = build_kernel()
    nc = _NC_CACHE

    in_maps = _host_prep(inputs)
    res = run_bass_kernel_spmd(nc, in_maps, core_ids=list(range(NCORES)))
    out = np.empty((B, L, DO), np.float32)
    for c in range(NCORES):
        # [128, NTILE*DO] -> [NTILE, 128, DO] -> [NLOC, DO]
        oc = res.results[c]["out"].reshape(128, NTILE, DO).transpose(1, 0, 2)
        out[c * BLOC:(c + 1) * BLOC] = (
            oc.astype(np.float32).reshape(BLOC, L, DO))
    return out
